# revision 14
# baseline (speedup 1.0000x reference)
"""Contrastive loss kernel for Trainium2 (8 NeuronCores, Bass/Tile).

Strategy
--------
Only rows with label==1 (pos) contribute losses, and only columns with
label==0 (neg) plus the diagonal enter each row's logsumexp.  The host
computes the tiny index sets from `labels`, then each of the 8 cores
(2 per batch) receives:
  gp: its half of the batch's positive greek rows      [P1, 256] f32
  ep: english rows at the same indices (for the diag)  [P1, 256] f32
  en: all negative english rows of the batch           [N1, 256] f32
padded with zero rows to the uniform compile-time shapes (P1, N1).

On device: L2-normalize rows (1/temperature folded into the greek
scale), cast bf16, PE-transpose to put H on partitions, matmul to get
logits in PSUM, then a single fused ScalarE pass exp(logit - 15) with
accumulate gives the per-row negative sums.  A fixed max constant (15 >
1/0.07) replaces the per-row max: logits are bounded so the logsumexp
stays exact in f32.  Zero-padded `en` rows yield *exactly* 0 logits, so
their exp(-15) contributions are removed with an exact scalar
correction.  Per-row loss = 15 + ln(exp(diag-15) + S + corr) - diag,
masked by a 0/1 weight vector and row-reduced; the host sums the 8x128
partials and divides by the positive count.
"""

import sys

if "/opt/trn_rl_repo" not in sys.path:
    sys.path.insert(0, "/opt/trn_rl_repo")

from contextlib import ExitStack

import ml_dtypes
import numpy as np

import concourse.bass as bass
import concourse.tile as tile
from concourse import mybir
from concourse.bass_utils import run_bass_kernel_spmd
from concourse.masks import make_identity

TEMPERATURE = 0.07
IGNORE_INDEX = -100
CMAX = 15.0
H = 256
N_CORES = 8

# Stash of the most recent BassKernelResults + shapes (for test harness timing).
LAST_RESULTS = None
LAST_SHAPES = None
TRACE = False


def _legalize_waits(nc: bass.Bass, max_waits: int = 1) -> None:
    """This container's walrus accepts at most one sync-wait per instruction
    (ACT structs especially); Tile can emit several.  Split the excess onto
    same-engine NoOps placed immediately before the instruction."""
    for bb in nc.main_func.blocks:
        new = []
        for ins in bb.instructions:
            si = ins.sync_info
            if si is not None and si.on_wait and len(si.on_wait) > max_waits:
                waits = list(si.on_wait)
                extra, keep = waits[:-max_waits], waits[-max_waits:]
                for i in range(0, len(extra), max_waits):
                    new.append(
                        mybir.InstNoOp(
                            name=nc.get_next_instruction_name(),
                            engine=ins.engine,
                            ins=[],
                            outs=[],
                            sync_info=mybir.SyncInfo(
                                on_wait=extra[i : i + max_waits], on_update=[]
                            ),
                            bass_nofuse=True,
                        )
                    )
                ins.sync_info = mybir.SyncInfo(
                    on_wait=keep, on_update=list(si.on_update or [])
                )
            new.append(ins)
        bb.instructions[:] = new


def _build_program(P1: int, N1: int, legalize: bool = True) -> bass.Bass:
    """One SPMD program: shapes P1 (pos rows) and N1 (neg rows) are uniform
    across cores; per-core data differs via in_maps."""
    PC = P1 // 128
    NC = N1 // 128
    NTILES = N1 // 512
    GROUPS = NC // 4  # 4-chunk transpose groups == 512-wide matmul slabs
    f32 = mybir.dt.float32
    bf16 = mybir.dt.bfloat16
    OP = mybir.AluOpType
    AF = mybir.ActivationFunctionType

    nc = bass.Bass()
    gp = nc.dram_tensor("gp", [P1, H], bf16, kind="ExternalInput")
    ep = nc.dram_tensor("ep", [P1, H], bf16, kind="ExternalInput")
    en = nc.dram_tensor("en", [N1, H], bf16, kind="ExternalInput")
    wv = nc.dram_tensor("wv", [P1], f32, kind="ExternalInput")
    corr = nc.dram_tensor("corr", [1, 1], f32, kind="ExternalInput")
    out = nc.dram_tensor("out", [128, 1], f32, kind="ExternalOutput")

    with tile.TileContext(nc) as tc, ExitStack() as ctx:
        persist = ctx.enter_context(tc.tile_pool(name="persist", bufs=1))
        small = ctx.enter_context(tc.tile_pool(name="small", bufs=1))
        scratch = ctx.enter_context(tc.tile_pool(name="scratch", bufs=3))
        expool = ctx.enter_context(tc.tile_pool(name="expool", bufs=2))
        psum_tp = ctx.enter_context(tc.tile_pool(name="psum_tp", bufs=2, space="PSUM"))
        psum_mm = ctx.enter_context(tc.tile_pool(name="psum_mm", bufs=2, space="PSUM"))

        # ---- constants (gpsimd: otherwise idle) + ACT table preload
        LOG_INV_T = float(-np.log(np.float64(TEMPERATURE)))
        eps_t = small.tile([128, 1], f32)
        nc.gpsimd.memset(eps_t[:], 1e-24)
        blnt_t = small.tile([128, 1], f32)
        nc.gpsimd.memset(blnt_t[:], LOG_INV_T)
        cneg_t = small.tile([128, 1], f32)
        nc.gpsimd.memset(cneg_t[:], -CMAX)
        ident = small.tile([128, 128], bf16)
        make_identity(nc, ident[:])
        # Dummy Ln at t~0 absorbs the ~2.7us ACT table load during the DMAs.
        dummy = small.tile([128, 1], f32)
        nc.scalar.activation(
            out=dummy[:], in_=eps_t[:], func=AF.Ln, bias=eps_t[:, 0:1], scale=1.0
        )

        # ---- loads (bf16), split per 4-chunk piece across the DMA queues
        # (SP + ACT hardware DGE, gpsimd software DGE) so they run in
        # parallel and unblock the pipeline piece by piece.
        # partition i holds rows {c*128+i : c in range(chunks)}
        Gf = persist.tile([128, PC, H], bf16)
        nc.sync.dma_start(out=Gf[:], in_=gp[:].rearrange("(c p) h -> p c h", p=128))
        en_r = en[:].rearrange("(c p) h -> p c h", p=128)
        Np = []
        for g in range(GROUPS):
            t = persist.tile([128, 4, H], bf16, tag=f"np{g}", name=f"np{g}")
            eng = nc.scalar if g % 2 == 1 else nc.sync
            eng.dma_start(out=t[:], in_=en_r[:, g * 4 : (g + 1) * 4, :])
            Np.append(t)
        Ef = persist.tile([128, PC, H], bf16)
        nc.gpsimd.dma_start(out=Ef[:], in_=ep[:].rearrange("(c p) h -> p c h", p=128))
        wt = small.tile([128, PC], f32)
        nc.sync.dma_start(out=wt[:], in_=wv[:].rearrange("(c p) -> p c", p=128))
        corr_t = small.tile([128, 1], f32)
        nc.sync.dma_start(out=corr_t[:], in_=corr[:].to_broadcast([128, 1]))

        # ---- row sums of squares (per 128-row chunk), piece-granular for e
        ssn = []
        for g in range(GROUPS):
            t = small.tile([128, 4], f32, tag=f"ssn{g}", name=f"ssn{g}")
            ssn.append(t)
        ssg = small.tile([128, PC], f32)
        sse = small.tile([128, PC], f32)

        def norm_jobs(xf, c, ss, sc):
            sq = scratch.tile([128, H], bf16, tag="sq")
            nc.vector.scalar_tensor_tensor(
                out=sq[:],
                in0=xf[:, c, :],
                scalar=1.0,
                in1=xf[:, c, :],
                op0=OP.mult,
                op1=OP.mult,
                accum_out=ss[:, sc : sc + 1],
            )

        def scale_of(ss, b):
            # rsqrt as exp(-0.5*ln(ss+eps)): one ACT table set for ln+exp.
            # eps=1e-24 matches the reference's clip(norm, 1e-12).
            nc.scalar.activation(
                out=ss[:], in_=ss[:], func=AF.Ln, bias=eps_t[:, 0:1], scale=1.0
            )
            bias = b if isinstance(b, float) else b[:, 0:1]
            nc.scalar.activation(out=ss[:], in_=ss[:], func=AF.Exp, bias=bias, scale=-0.5)

        # greek norms first (its chain ends at the matmul stationary side),
        # then the e pieces in arrival order
        for c in range(PC):
            norm_jobs(Gf, c, ssg, c)
        scale_of(ssg, blnt_t)  # greek scale carries the 1/T
        for g in range(GROUPS):
            for c in range(4):
                norm_jobs(Np[g], c, ssn[g], c)
            scale_of(ssn[g], 0.0)

        # ---- apply scales -> bf16 matmul operands, on the idle gpsimd
        Gb = persist.tile([128, PC, H], bf16)
        for c in range(PC):
            nc.gpsimd.tensor_scalar_mul(Gb[:, c, :], Gf[:, c, :], ssg[:, c : c + 1])
        Nb = []
        for g in range(GROUPS):
            t = persist.tile([128, 4, H], bf16, tag=f"nb{g}", name=f"nb{g}")
            for c in range(4):
                nc.gpsimd.tensor_scalar_mul(t[:, c, :], Np[g][:, c, :], ssn[g][:, c : c + 1])
            Nb.append(t)

        # ---- transpose to put H on partitions (PE) + copy PSUM->SBUF (DVE)
        GbT = persist.tile([128, 2, P1], bf16)
        for c0 in range(0, PC, 4):
            cn = min(4, PC - c0)
            for hk in range(2):
                pt = psum_tp.tile([128, 512], bf16, tag="pt")
                for j in range(cn):
                    nc.tensor.transpose(
                        pt[:, j * 128 : (j + 1) * 128],
                        Gb[:, c0 + j, hk * 128 : (hk + 1) * 128],
                        ident[:],
                    )
                nc.scalar.copy(
                    out=GbT[:, hk, c0 * 128 : (c0 + cn) * 128], in_=pt[:, : cn * 128]
                )
        NbT = [
            persist.tile([128, 2, 512], bf16, tag=f"nbt{g}", name=f"nbt{g}")
            for g in range(GROUPS)
        ]
        for g in range(GROUPS):
            for hk in range(2):
                pt = psum_tp.tile([128, 512], bf16, tag="pt")
                for j in range(4):
                    nc.tensor.transpose(
                        pt[:, j * 128 : (j + 1) * 128],
                        Nb[g][:, j, hk * 128 : (hk + 1) * 128],
                        ident[:],
                    )
                nc.vector.tensor_copy(out=NbT[g][:, hk, :], in_=pt[:])

        # ---- logits + one fused exp/accumulate pass per 128-row chunk
        # S[p, c] = sum_q exp(logit[c*128+p, q] - CMAX)
        S = small.tile([128, PC], f32)
        for c in range(PC):
            pm = psum_mm.tile([128, N1], f32, tag="pm")
            for nt in range(NTILES):
                for hk in range(2):
                    nc.tensor.matmul(
                        pm[:, nt * 512 : (nt + 1) * 512],
                        GbT[:, hk, c * 128 : (c + 1) * 128],
                        NbT[nt][:, hk, :],
                        start=(hk == 0),
                        stop=(hk == 1),
                    )
            ex = expool.tile([128, N1], f32, tag="ex")
            nc.scalar.activation(
                out=ex[:],
                in_=pm[:],
                func=AF.Exp,
                bias=cneg_t[:, 0:1],
                scale=1.0,
                accum_out=S[:, c : c + 1],
            )

        # ---- diag[p] = raw greek.english dot, scaled by both row norms
        for c in range(PC):
            norm_jobs(Ef, c, sse, c)
        scale_of(sse, 0.0)
        diag = small.tile([128, PC], f32)
        for c in range(PC):
            dsq = scratch.tile([128, H], bf16, tag="dsq")
            nc.vector.scalar_tensor_tensor(
                out=dsq[:],
                in0=Gf[:, c, :],
                scalar=1.0,
                in1=Ef[:, c, :],
                op0=OP.mult,
                op1=OP.mult,
                accum_out=diag[:, c : c + 1],
            )
        nc.vector.tensor_mul(diag[:], diag[:], ssg[:])
        nc.vector.tensor_mul(diag[:], diag[:], sse[:])

        # ---- per-row loss and masked partial sum
        ed = small.tile([128, PC], f32)
        nc.scalar.activation(
            out=ed[:], in_=diag[:], func=AF.Exp, bias=cneg_t[:, 0:1], scale=1.0
        )
        t2 = small.tile([128, PC], f32)
        nc.vector.scalar_tensor_tensor(
            out=t2[:],
            in0=S[:],
            scalar=corr_t[:, 0:1],
            in1=ed[:],
            op0=OP.add,
            op1=OP.add,
        )
        nc.scalar.activation(out=t2[:], in_=t2[:], func=AF.Ln)
        # loss = (ln(...) + CMAX) - diag
        loss = small.tile([128, PC], f32)
        nc.vector.scalar_tensor_tensor(
            out=loss[:],
            in0=t2[:],
            scalar=CMAX,
            in1=diag[:],
            op0=OP.add,
            op1=OP.subtract,
        )
        lm = small.tile([128, PC], f32)
        part = small.tile([128, 1], f32)
        nc.vector.scalar_tensor_tensor(
            out=lm[:],
            in0=loss[:],
            scalar=1.0,
            in1=wt[:],
            op0=OP.mult,
            op1=OP.mult,
            accum_out=part[:],
        )
        nc.sync.dma_start(out=out[:], in_=part[:])
    if legalize:
        _legalize_waits(nc, max_waits=1)
    return nc


def _pad_rows(x: np.ndarray, n: int) -> np.ndarray:
    outp = np.zeros((n,) + x.shape[1:], dtype=x.dtype)
    outp[: x.shape[0]] = x
    return outp


def kernel(greek_embeds, english_embeds, labels):
    global LAST_RESULTS
    g = np.ascontiguousarray(np.asarray(greek_embeds, dtype=np.float32))
    e = np.ascontiguousarray(np.asarray(english_embeds, dtype=np.float32))
    lab = np.asarray(labels)
    B, P, Hh = g.shape
    assert Hh == H and B * 2 == N_CORES

    valid = lab != IGNORE_INDEX
    pos = valid & (lab == 1)
    neg = valid & (lab != 1)
    ok = (valid.sum(-1) >= 2) & pos.any(-1) & neg.any(-1)

    count = int(pos[ok].sum()) if ok.any() else 0
    if count == 0:
        return np.float32(0.0)

    pos_idx = [np.nonzero(pos[b])[0] if ok[b] else np.zeros(0, np.int64) for b in range(B)]
    neg_idx = [np.nonzero(neg[b])[0] if ok[b] else np.zeros(0, np.int64) for b in range(B)]
    halves = [np.array_split(pi, 2) for pi in pos_idx]

    np_max = max(len(halves[b][h]) for b in range(B) for h in range(2))
    nn_max = max(len(ni) for ni in neg_idx)
    P1 = max(128, ((np_max + 127) // 128) * 128)
    N1 = max(512, ((nn_max + 511) // 512) * 512)

    E15 = np.float32(np.exp(np.float32(-CMAX)))
    in_maps = []
    for core in range(N_CORES):
        b, hf = core // 2, core % 2
        p_idx = halves[b][hf]
        n_idx = neg_idx[b]
        w = np.zeros(P1, np.float32)
        w[: len(p_idx)] = 1.0
        in_maps.append(
            {
                "gp": _pad_rows(g[b][p_idx].astype(ml_dtypes.bfloat16), P1),
                "ep": _pad_rows(e[b][p_idx].astype(ml_dtypes.bfloat16), P1),
                "en": _pad_rows(e[b][n_idx].astype(ml_dtypes.bfloat16), N1),
                "wv": w,
                "corr": np.array([[-(N1 - len(n_idx)) * float(E15)]], np.float32),
            }
        )

    global LAST_SHAPES
    LAST_SHAPES = (P1, N1, dict(in_maps[0]))
    nc = _build_program(P1, N1)
    res = run_bass_kernel_spmd(nc, in_maps, list(range(N_CORES)), trace=TRACE)
    LAST_RESULTS = res
    total = sum(float(r["out"].sum()) for r in res.results)
    return np.float32(total / count)
